# revision 15
# baseline (speedup 1.0000x reference)
"""Trainium2 Bass kernel for nn_FPSWE_40303973105696.

Computation (see problem reference): project X onto P directions, sort along
N, linearly interpolate N->M quantiles, subtract from ref, contract with
weight.

Algebraic folding done on host:
    out[b, p] = rw[p] - sum_n Xs[b, n, p] * W2[p, n]
where
    rw[p]    = sum_m ref[m] * weight[p, m]
    W2[p, n] = interpolation matrix folded into weight (fixed scatter)
    Xs       = sort_n(X @ theta_w.T)

Device kernel per core (data-parallel over B, core c handles batch c):
    1. proj[p, n] = theta_w[p, :] @ X[b].T        (PE, fp32)
    2. sort proj rows along n (free axis)         (bitonic, 66 stages)
    3. acc[p] = sum_n Xs[p, n] * W2[p, n]         (DVE tensor_tensor_reduce)
    4. out[p] = rw[p] - acc[p]

The sort keeps row-groups in wide SBUF buffers [128, G*2048] so each
compare-exchange stage is a single min + single max tensor_tensor op over the
whole group (blocks tile uniformly because 2048 % block == 0).
"""

import numpy as np

from concourse import bass, bacc, mybir
from concourse.tile import TileContext
from concourse.bass_utils import run_bass_kernel_spmd

B, N, D, P, M = 8, 2048, 128, 1024, 1024
NT = P // 128          # 8 projection row-chunks of 128 partitions each
MM_CHUNK = 512         # matmul free-dim chunk (one PSUM bank)
N_CORES = 8

# row-groups: list of (num_row_chunks, engine_name). Sum must be NT.
GROUPS = [(NT, "vector")]

# debug knob: limit number of sort stages emitted (None = all)
STAGE_LIMIT = None

FP = mybir.dt.float32


def _sort_stages(n):
    """Uniform-direction bitonic network: (kind, param) list. 66 stages for n=2048."""
    stages = []
    size = 2
    while size <= n:
        stages.append(("mirror", size))
        st = size // 4
        while st >= 1:
            stages.append(("std", st))
            st //= 2
        size *= 2
    return stages


def _emit_sort_stage(eng, kind, val, cur, oth):
    """One compare-exchange stage: read cur AP, write oth AP (same shape)."""
    if kind == "mirror":
        s = val
        half = s // 2
        v = cur.rearrange("p (n s) -> p n s", s=s)
        o = oth.rearrange("p (n s) -> p n s", s=s)
        lo, up = v[:, :, :half], v[:, :, half:]
        olo, oup = o[:, :, :half], o[:, :, half:]
        eng.tensor_tensor(olo, lo, up[:, :, ::-1], op=mybir.AluOpType.min)
        eng.tensor_tensor(oup, lo[:, :, ::-1], up, op=mybir.AluOpType.max)
    else:
        st = val
        v = cur.rearrange("p (n t s) -> p n t s", t=2, s=st)
        o = oth.rearrange("p (n t s) -> p n t s", t=2, s=st)
        eng.tensor_tensor(o[:, :, 0, :], v[:, :, 0, :], v[:, :, 1, :],
                          op=mybir.AluOpType.min)
        eng.tensor_tensor(o[:, :, 1, :], v[:, :, 0, :], v[:, :, 1, :],
                          op=mybir.AluOpType.max)


def _build_kernel():
    assert sum(g for g, _ in GROUPS) == NT
    nc = bacc.Bacc()

    xt = nc.declare_dram_parameter("xt", [D, N], FP, isOutput=False)       # X[b].T
    tht = nc.declare_dram_parameter("tht", [D, P], FP, isOutput=False)     # theta_w.T
    w2 = nc.declare_dram_parameter("w2", [P, N], FP, isOutput=False)       # folded weight
    rw = nc.declare_dram_parameter("rw", [128, NT], FP, isOutput=False)    # rw[p] as [128, 8]
    out = nc.declare_dram_parameter("out", [128, NT], FP, isOutput=True)

    stages = _sort_stages(N)
    if STAGE_LIMIT is not None:
        stages = stages[:STAGE_LIMIT]

    with TileContext(nc) as tc:
        with (
            tc.tile_pool(name="const", bufs=1) as const_pool,
            tc.tile_pool(name="xt", bufs=1) as xt_pool,
            tc.tile_pool(name="sa", bufs=1) as a_pool,
            tc.tile_pool(name="sb", bufs=1) as b_pool,
            tc.tile_pool(name="w2", bufs=3) as w2_pool,
            tc.tile_pool(name="ps", bufs=2, space="PSUM") as psum_pool,
        ):
            tht_sb = const_pool.tile([D, P], FP, tag="tht")
            tht_raw = const_pool.tile([D, P], FP, tag="thtr")
            rw_sb = const_pool.tile([128, NT], FP, tag="rw")
            acc_sb = const_pool.tile([128, NT], FP, tag="acc")
            out_sb = const_pool.tile([128, NT], FP, tag="outsb")
            xt_sb = xt_pool.tile([D, N], FP, tag="xt")
            xt_raw = xt_pool.tile([D, N], FP, tag="xtr")

            # Bounce DMA'd matmul inputs through ACT so Matmult instructions
            # never carry two DMA-queue semaphore waits (walrus codegen limit).
            nc.sync.dma_start(out=tht_raw[:], in_=tht[:])
            nc.sync.dma_start(out=rw_sb[:], in_=rw[:])
            nc.sync.dma_start(out=xt_raw[:], in_=xt[:])
            nc.scalar.copy(out=tht_sb[:], in_=tht_raw[:])
            nc.scalar.copy(out=xt_sb[:], in_=xt_raw[:])

            t0 = 0
            for gi, (gsz, eng_name) in enumerate(GROUPS):
                a_t = a_pool.tile([128, gsz * N], FP, tag=f"a{gi}", name=f"a{gi}")
                b_t = b_pool.tile([128, gsz * N], FP, tag=f"b{gi}", name=f"b{gi}")
                eng = getattr(nc, eng_name)

                # ---- projection matmuls for this group's row chunks ----
                for r in range(gsz):
                    t = t0 + r
                    ps = psum_pool.tile([128, N], FP, tag="ps", name="ps")
                    for ch in range(N // MM_CHUNK):
                        nc.tensor.matmul(
                            ps[:, ch * MM_CHUNK:(ch + 1) * MM_CHUNK],
                            lhsT=tht_sb[:, t * 128:(t + 1) * 128],
                            rhs=xt_sb[:, ch * MM_CHUNK:(ch + 1) * MM_CHUNK],
                            start=True, stop=True,
                        )
                    nc.scalar.copy(out=a_t[:, r * N:(r + 1) * N], in_=ps[:])

                # ---- bitonic sort along free axis (ends back in a_t) ----
                cur, oth = a_t[:], b_t[:]
                for kind, val in stages:
                    _emit_sort_stage(eng, kind, val, cur, oth)
                    cur, oth = oth, cur

                # ---- weighted reduction per row chunk ----
                for r in range(gsz):
                    t = t0 + r
                    w2_sb = w2_pool.tile([128, N], FP, tag="w2", name="w2")
                    nc.sync.dma_start(out=w2_sb[:], in_=w2[t * 128:(t + 1) * 128, :])
                    scratch = oth[:, r * N:(r + 1) * N]  # dead ping buffer
                    nc.vector.tensor_mul(scratch, cur[:, r * N:(r + 1) * N], w2_sb[:])
                    nc.vector.reduce_sum(acc_sb[:, t:t + 1], scratch,
                                         axis=mybir.AxisListType.X)
                t0 += gsz

            nc.vector.tensor_sub(out_sb[:], rw_sb[:], acc_sb[:])
            nc.sync.dma_start(out=out[:], in_=out_sb[:])

    return nc


_NC_CACHE = None


def _get_nc():
    global _NC_CACHE
    if _NC_CACHE is None:
        nc = _build_kernel()
        nc.finalize()   # Bacc: runs wait-splitting + register allocation
        _NC_CACHE = nc
    return _NC_CACHE


def _host_precompute(X, theta_w, ref, weight):
    X = np.ascontiguousarray(np.asarray(X, dtype=np.float32))
    theta_w = np.asarray(theta_w, dtype=np.float32)
    ref = np.asarray(ref, dtype=np.float32)
    weight = np.asarray(weight, dtype=np.float32)

    xt = np.ascontiguousarray(X.transpose(0, 2, 1))          # [B, D, N]
    tht = np.ascontiguousarray(theta_w.T)                    # [D, P]

    x1d = np.linspace(0.0, 1.0, N + 2, dtype=np.float32)[1:-1]
    xnew = np.linspace(0.0, 1.0, M + 2, dtype=np.float32)[1:-1]
    ind = np.clip(np.searchsorted(x1d, xnew) - 1, 0, N - 2)
    eps = np.float32(np.finfo(np.float32).eps)
    dx = x1d[1:] - x1d[:-1]
    t = ((xnew - x1d[ind]) / (eps + dx[ind])).astype(np.float64)

    w2nt = np.zeros((N, P), dtype=np.float64)                # [N, P]
    wT = weight.T.astype(np.float64)                         # [M, P]
    np.add.at(w2nt, ind, (1.0 - t)[:, None] * wT)
    np.add.at(w2nt, ind + 1, t[:, None] * wT)
    w2 = np.ascontiguousarray(w2nt.T.astype(np.float32))     # [P, N]

    rw = (weight.astype(np.float64) @ ref.astype(np.float64)).astype(np.float32)
    rw_sb = np.ascontiguousarray(rw.reshape(NT, 128).T)      # [128, NT]
    return xt, tht, w2, rw_sb


def _in_maps(X, theta_w, ref, weight):
    xt, tht, w2, rw_sb = _host_precompute(X, theta_w, ref, weight)
    return [
        {"xt": xt[c], "tht": tht, "w2": w2, "rw": rw_sb}
        for c in range(N_CORES)
    ]


def kernel(X, theta_w, ref, weight):
    nc = _get_nc()
    res = run_bass_kernel_spmd(nc, _in_maps(X, theta_w, ref, weight),
                               list(range(N_CORES)))
    outs = res.results if hasattr(res, "results") else res
    out_full = np.empty((B, P), dtype=np.float32)
    for c in range(N_CORES):
        out_full[c] = np.ascontiguousarray(outs[c]["out"].T).reshape(P)
    return out_full


# ---------------------------------------------------------------------------
# Benchmark path: cached jit + device-resident inputs, excludes host transfer.
# ---------------------------------------------------------------------------

def make_bench(X, theta_w, ref, weight):
    import jax
    from jax.sharding import Mesh, PartitionSpec
    from jax.experimental.shard_map import shard_map
    from concourse import bass2jax, mybir as _mybir
    from concourse.bass2jax import (
        _bass_exec_p, install_neuronx_cc_hook, partition_id_tensor,
    )

    install_neuronx_cc_hook()
    nc = _get_nc()
    in_maps = _in_maps(X, theta_w, ref, weight)

    partition_name = (nc.partition_id_tensor.name
                      if nc.partition_id_tensor else None)
    in_names, out_names, out_avals, zero_outs = [], [], [], []
    for alloc in nc.m.functions[0].allocations:
        if not isinstance(alloc, _mybir.MemoryLocationSet):
            continue
        name = alloc.memorylocations[0].name
        if alloc.kind == "ExternalInput":
            if name == partition_name:
                continue
            in_names.append(name)
        elif alloc.kind == "ExternalOutput":
            out_names.append(name)
            shape = tuple(alloc.tensor_shape)
            dtype = _mybir.dt.np(alloc.dtype)
            out_avals.append(jax.core.ShapedArray(shape, dtype))
            zero_outs.append(np.zeros(shape, dtype))
    n_params = len(in_names)
    all_names = in_names + out_names
    if partition_name is not None:
        all_names = all_names + [partition_name]

    def _body(*args):
        operands = list(args)
        if partition_name is not None:
            operands.append(partition_id_tensor())
        outs = _bass_exec_p.bind(
            *operands,
            out_avals=tuple(out_avals),
            in_names=tuple(all_names),
            out_names=tuple(out_names),
            lowering_input_output_aliases=(),
            sim_require_finite=True,
            sim_require_nnan=True,
            nc=nc,
        )
        return tuple(outs)

    devices = jax.devices()[:N_CORES]
    mesh = Mesh(np.asarray(devices), ("core",))
    nin = n_params + len(zero_outs)
    fn = jax.jit(
        shard_map(_body, mesh=mesh,
                  in_specs=(PartitionSpec("core"),) * nin,
                  out_specs=(PartitionSpec("core"),) * len(out_names),
                  check_rep=False),
        keep_unused=True,
    )
    per_core = [[np.asarray(m[nm]) for nm in in_names] for m in in_maps]
    concat_in = [
        np.concatenate([per_core[c][i] for c in range(N_CORES)], axis=0)
        for i in range(n_params)
    ] + [
        np.concatenate([z for _ in range(N_CORES)], axis=0) for z in zero_outs
    ]
    dev_in = [jax.device_put(a) for a in concat_in]

    def run():
        outs = fn(*dev_in)
        jax.block_until_ready(outs)
        return outs

    def collect(outs):
        arrs = [np.asarray(o) for o in outs]
        out_full = np.empty((B, P), dtype=np.float32)
        o = arrs[0]  # [8*128, NT]
        for c in range(N_CORES):
            out_full[c] = np.ascontiguousarray(o[c * 128:(c + 1) * 128].T).reshape(P)
        return out_full

    return run, collect
